# revision 4
# baseline (speedup 1.0000x reference)
"""Trainium2 Bass kernel for nn_CrossAttention (B=8, C=1024, L=4096, CTX=768).

Math: with a single context position per batch, the module collapses:
  q = Wq@gn(x), s[l] = q[:,l]·k  ==>  s[l] = (Wq^T k)·gn(x)[:,l]
  out_proj = Wp @ (v ⊗ attn)    ==>  (Wp v) ⊗ attn          (rank-1)
GroupNorm affine + softmax shift invariance reduce the score to
  s[l] = Σ_g rsig_g · t[g,l],   t[g,l] = Σ_{c∈g} (u·γ)[c] x[c,l]
(all additive constants drop inside softmax). So per batch the device only
needs: group stats of x, the 8 per-group partial dots t, a softmax over L,
and out = x + pv ⊗ attn + bp.  The tiny context-side algebra (layernorm,
kv/u/pv matvecs) is host-side preprocessing during input sharding.

Sharding: data-parallel over batch B — one batch per NeuronCore, 8 cores,
no collectives. Per-core HBM traffic = 16 MiB in + 16 MiB out.

Layout: x lives SBUF-resident as [p=128, g=8, l=4096] (channel c = g*128+p).
x arrives in 8 L-strips so the per-strip t matmuls can run a short PSUM
accumulation over all 8 groups; a block-diagonal lhsT (ugmask[:, m, g] =
ug_g iff m==g) routes group g's partial dot to PSUM row g with base
partition 0 (matmul PSUM outputs must start at partition 0/32/64/96).
"""

import numpy as np

import concourse.bacc as bacc
import concourse.tile as tile
import concourse.mybir as mybir
from concourse.bass_utils import run_bass_kernel_spmd

B, C, L = 8, 1024, 4096
CTX = 768
G = 8            # groups; group g == channel chunk [g*128, (g+1)*128)
GP = C // G      # 128 channels per group == SBUF partitions
LT = 8           # L strips
LTS = L // LT    # 512
EPS = 1e-5
SCALE = float(C) ** -0.5   # head_dim == C

F32 = mybir.dt.float32
_CACHE = {}


def _build():
    nc = bacc.Bacc("TRN2", target_bir_lowering=False, debug=False, num_devices=B)

    x_d = nc.dram_tensor("x", [C, L], F32, kind="ExternalInput").ap()
    ugm_d = nc.dram_tensor("ugmask", [GP, G, G], F32, kind="ExternalInput").ap()
    pv_d = nc.dram_tensor("pvr", [C], F32, kind="ExternalInput").ap()
    bp_d = nc.dram_tensor("bpc", [GP, G], F32, kind="ExternalInput").ap()
    out_d = nc.dram_tensor("out", [C, L], F32, kind="ExternalOutput").ap()

    # HBM view: channel c = g*GP + p  ->  [p, g, l]
    x_src = x_d.rearrange("(g p) n -> p g n", p=GP)
    out_dst = out_d.rearrange("(g p) n -> p g n", p=GP)

    with tile.TileContext(nc) as tc:
        with (
            tc.tile_pool(name="singles", bufs=1) as singles,
            tc.tile_pool(name="xpool", bufs=1) as xpool,
            tc.tile_pool(name="obuf", bufs=4) as obuf,
            tc.tile_pool(name="t_pool", bufs=2, space="PSUM") as t_pool,
            tc.tile_pool(name="gs_pool", bufs=1, space="PSUM") as gs_pool,
            tc.tile_pool(name="s_pool", bufs=2, space="PSUM") as s_pool,
            tc.tile_pool(name="o_pool", bufs=3, space="PSUM") as o_pool,
        ):
            ugm_sb = singles.tile([GP, G, G], F32)     # block-diag (u*gamma)
            pv_sb = singles.tile([1, G, GP], F32)      # Wp@v rows per chunk
            bp_sb = singles.tile([GP, G], F32)         # out-proj bias, [p, g]
            nc.sync.dma_start(out=ugm_sb[:], in_=ugm_d)
            nc.sync.dma_start(out=pv_sb[0:1, :, :], in_=pv_d[None, :])
            nc.sync.dma_start(out=bp_sb[:], in_=bp_d)

            # eye(G) ⊗ 1/GP : routes chunk-g stat means to PSUM row g
            invm = singles.tile([GP, G, G], F32)
            nc.vector.memset(invm[:], 0.0)
            for g in range(G):
                nc.vector.memset(invm[:, g, g:g + 1], 1.0 / GP)
            eps8 = singles.tile([G, 1], F32)
            nc.vector.memset(eps8[:], EPS)

            x_sb = xpool.tile([GP, G, L], F32)         # full batch resident
            st6 = singles.tile([GP, G, LT, 6], F32)    # bn_stats out
            me = singles.tile([GP, G, 2], F32)         # per-partition mean/var
            me2 = singles.tile([GP, G, 2], F32)        # (mean, E[x^2])
            t_sb = singles.tile([G, LT, LTS], F32)     # t[g, l]

            gs_ps = gs_pool.tile([G, 2], F32)          # per-group mean, E[x^2]

            for lt in range(LT):
                sl = slice(lt * LTS, (lt + 1) * LTS)
                nc.sync.dma_start(out=x_sb[:, :, sl], in_=x_src[:, :, sl])
                t_ps = t_pool.tile([G, LTS], F32)
                for g in range(G):
                    nc.vector.bn_stats(out=st6[:, g, lt, :], in_=x_sb[:, g, sl])
                    nc.tensor.matmul(
                        t_ps[:], lhsT=ugm_sb[:, :, g], rhs=x_sb[:, g, sl],
                        start=(g == 0), stop=(g == G - 1),
                    )
                nc.scalar.copy(t_sb[:, lt, :], t_ps[:])

            # per-partition -> per-group stats
            for g in range(G):
                nc.vector.bn_aggr(out=me[:, g, :], in_=st6[:, g, :, :])
                nc.vector.tensor_copy(me2[:, g, 0:1], me[:, g, 0:1])
                nc.vector.tensor_mul(me2[:, g, 1:2], me[:, g, 0:1], me[:, g, 0:1])
                nc.vector.tensor_add(me2[:, g, 1:2], me[:, g, 1:2], me2[:, g, 1:2])
                nc.tensor.matmul(gs_ps[:], lhsT=invm[:, :, g], rhs=me2[:, g, :],
                                 start=(g == 0), stop=(g == G - 1))

            # rsig_g = 1/sqrt(E[x^2] - mu^2 + eps)  on 8 partitions
            gs_sb = singles.tile([G, 2], F32)
            mu2 = singles.tile([G, 1], F32)
            varg = singles.tile([G, 1], F32)
            sd8 = singles.tile([G, 1], F32)
            rsig8 = singles.tile([G, 1], F32)
            nc.scalar.copy(gs_sb[:], gs_ps[:])
            nc.vector.tensor_mul(mu2[:], gs_sb[:, 0:1], gs_sb[:, 0:1])
            nc.vector.tensor_sub(varg[:], gs_sb[:, 1:2], mu2[:])
            nc.scalar.activation(sd8[:], varg[:], mybir.ActivationFunctionType.Sqrt,
                                 bias=eps8[:], scale=1.0)
            nc.vector.reciprocal(rsig8[:], sd8[:])

            # s[l] = Σ_g rsig_g t[g,l]; e = exp(s/32); Z = Σ e; attn = e/Z
            erow = singles.tile([1, L], F32)
            zrow = singles.tile([1, LT], F32)
            for lt in range(LT):
                s_ps = s_pool.tile([1, LTS], F32)
                nc.tensor.matmul(s_ps[:], lhsT=rsig8[:], rhs=t_sb[:, lt, :],
                                 start=True, stop=True)
                nc.scalar.activation(erow[0:1, lt * LTS:(lt + 1) * LTS], s_ps[:],
                                     mybir.ActivationFunctionType.Exp,
                                     scale=SCALE, accum_out=zrow[0:1, lt:lt + 1])
            z1 = singles.tile([1, 1], F32)
            rz = singles.tile([1, 1], F32)
            attn = singles.tile([1, L], F32)
            nc.vector.reduce_sum(out=z1[:], in_=zrow[:], axis=mybir.AxisListType.X)
            nc.vector.reciprocal(rz[:], z1[:])
            nc.vector.tensor_scalar_mul(attn[:], erow[:], rz[0:1, 0:1])

            # out = x + pv ⊗ attn + bp
            for g in range(G):
                for lt in range(LT):
                    sl = slice(lt * LTS, (lt + 1) * LTS)
                    o_ps = o_pool.tile([GP, LTS], F32)
                    nc.tensor.matmul(o_ps[:], lhsT=pv_sb[0:1, g, :], rhs=attn[0:1, sl],
                                     start=True, stop=True)
                    o_sb = obuf.tile([GP, LTS], F32)
                    nc.vector.scalar_tensor_tensor(
                        out=o_sb[:], in0=o_ps[:], scalar=bp_sb[:, g:g + 1],
                        in1=x_sb[:, g, sl],
                        op0=mybir.AluOpType.add, op1=mybir.AluOpType.add,
                    )
                    nc.sync.dma_start(out=out_dst[:, g, sl], in_=o_sb[:])

    nc.compile()
    return nc


def kernel(x, context, gn_gamma, gn_beta, ln_gamma, ln_beta,
           Wq, bq, Wkv, bkv, Wp, bp):
    x = np.asarray(x, np.float32)
    ctx = np.asarray(context, np.float64)
    # host-side context path (tiny): layernorm -> kv -> u = Wq^T k, pv = Wp v
    mu = ctx.mean(axis=1, keepdims=True)
    var = ctx.var(axis=1, keepdims=True)
    ctxn = (ctx - mu) / np.sqrt(var + EPS) * np.asarray(ln_gamma, np.float64) \
        + np.asarray(ln_beta, np.float64)
    kv = ctxn @ np.asarray(Wkv, np.float64).T + np.asarray(bkv, np.float64)
    k, v = kv[:, :C], kv[:, C:]
    u = k @ np.asarray(Wq, np.float64)               # u[b,c] = Σ_o k[b,o] Wq[o,c]
    ug = u * np.asarray(gn_gamma, np.float64)[None, :]
    pv = v @ np.asarray(Wp, np.float64).T            # pv[b,o] = Σ_c Wp[o,c] v[b,c]

    if "nc" not in _CACHE:
        _CACHE["nc"] = _build()
    nc = _CACHE["nc"]

    bpc = np.ascontiguousarray(
        np.asarray(bp, np.float32).reshape(G, GP).T)          # [p, g]
    in_maps = []
    for b in range(B):
        ugb = ug[b].astype(np.float32).reshape(G, GP)          # [g, p]
        ugmask = np.zeros((GP, G, G), np.float32)              # [p, m, g]
        for g in range(G):
            ugmask[:, g, g] = ugb[g]
        in_maps.append({
            "x": np.ascontiguousarray(x[b]),
            "ugmask": ugmask,
            "pvr": np.ascontiguousarray(pv[b].astype(np.float32)),
            "bpc": bpc,
        })
    global _LAST_IN_MAPS
    _LAST_IN_MAPS = in_maps
    res = run_bass_kernel_spmd(nc, in_maps, list(range(B)))
    return np.stack([res.results[b]["out"] for b in range(B)]).astype(np.float32)


# revision 7
# speedup vs baseline: 1.1772x; 1.1772x over previous
"""Trainium2 Bass kernel for nn_CrossAttention (B=8, C=1024, L=4096, CTX=768).

Math: with a single context position per batch, the module collapses:
  q = Wq@gn(x), s[l] = q[:,l]*k  ==>  s[l] = (Wq^T k)*gn(x)[:,l]
  out_proj = Wp @ (v x attn)    ==>  (Wp v) x attn          (rank-1)
GroupNorm affine + softmax shift invariance reduce the score to
  s[l] = sum_g rsig_g * t[g,l],   t[g,l] = sum_{c in g} (u*gamma)[c] x[c,l]
(all additive constants drop inside softmax). So per batch the device only
needs: group stats of x, the 8 per-group partial dots t, a softmax over L,
and out = x + pv x attn + bp.  The tiny context-side algebra (layernorm,
kv/u/pv matvecs) is host-side preprocessing during input sharding.

Sharding: data-parallel over batch B -- one batch per NeuronCore, 8 cores,
no collectives. Per-core HBM traffic = 16 MiB in + 16 MiB out.

Layout: x lives SBUF-resident in exact fp32 as [p=128, g=8, l=4096]
(channel c = g*128+p); the residual and group stats read it exactly.
x arrives in 8 L-strips; each strip is also cast fp32->bf16 by a gpsimd
(SWDGE) SBUF->SBUF DMA so the score matmuls run at the PE's full bf16
column rate (fp32 matmul is 4x slower). The scores only weight a ~1e-4
magnitude additive term, so bf16 there perturbs the output by ~5e-7.
Per-strip t matmuls accumulate over the 8 groups in PSUM; a block-diagonal
lhsT (ugmask[:, m, g] = ug_g iff m==g) routes group g's partial dot to
PSUM row g with base partition 0 (matmul PSUM outputs must start at
partition 0/32/64/96, and lhsT/rhs must share a base partition).
"""

import numpy as np
import ml_dtypes

import concourse.bacc as bacc
import concourse.tile as tile
import concourse.mybir as mybir
from concourse.bass_utils import run_bass_kernel_spmd

B, C, L = 8, 1024, 4096
CTX = 768
G = 8            # groups; group g == channel chunk [g*128, (g+1)*128)
GP = C // G      # 128 channels per group == SBUF partitions
LT = 8           # L strips
LTS = L // LT    # 512
EPS = 1e-5
SCALE = float(C) ** -0.5   # head_dim == C

F32 = mybir.dt.float32
BF16 = mybir.dt.bfloat16
_CACHE = {}


def _build():
    nc = bacc.Bacc("TRN2", target_bir_lowering=False, debug=False, num_devices=B)

    x_d = nc.dram_tensor("x", [C, L], F32, kind="ExternalInput").ap()
    ugm_d = nc.dram_tensor("ugmask", [GP, G, G], BF16, kind="ExternalInput").ap()
    pv_d = nc.dram_tensor("pvr", [C], BF16, kind="ExternalInput").ap()
    bp_d = nc.dram_tensor("bpc", [GP, G], F32, kind="ExternalInput").ap()
    out_d = nc.dram_tensor("out", [C, L], F32, kind="ExternalOutput").ap()

    # HBM view: channel c = g*GP + p  ->  [p, g, l]
    x_src = x_d.rearrange("(g p) n -> p g n", p=GP)
    out_dst = out_d.rearrange("(g p) n -> p g n", p=GP)

    with tile.TileContext(nc) as tc:
        with (
            tc.tile_pool(name="singles", bufs=1) as singles,
            tc.tile_pool(name="xpool", bufs=1) as xpool,
            tc.tile_pool(name="xb16pool", bufs=3) as xb16pool,
            tc.tile_pool(name="obuf", bufs=4) as obuf,
            tc.tile_pool(name="t_pool", bufs=2, space="PSUM") as t_pool,
            tc.tile_pool(name="gs_pool", bufs=1, space="PSUM") as gs_pool,
            tc.tile_pool(name="s_pool", bufs=2, space="PSUM") as s_pool,
            tc.tile_pool(name="o_pool", bufs=3, space="PSUM") as o_pool,
        ):
            ugm_sb = singles.tile([GP, G, G], BF16)    # block-diag (u*gamma)
            pv_sb = singles.tile([1, G, GP], BF16)     # Wp@v rows per chunk
            bp_sb = singles.tile([GP, G], F32)         # out-proj bias, [p, g]
            nc.sync.dma_start(out=ugm_sb[:], in_=ugm_d)
            nc.sync.dma_start(out=pv_sb[0:1, :, :], in_=pv_d[None, :])
            nc.sync.dma_start(out=bp_sb[:], in_=bp_d)

            # eye(G) (x) 1/GP : routes chunk-g stat means to PSUM row g
            invm = singles.tile([GP, G, G], F32)
            nc.vector.memset(invm[:], 0.0)
            for g in range(G):
                nc.vector.memset(invm[:, g, g:g + 1], 1.0 / GP)
            eps8 = singles.tile([G, 1], F32)
            nc.vector.memset(eps8[:], EPS)

            x_sb = xpool.tile([GP, G, L], F32)         # full batch resident
            st6 = singles.tile([GP, G, LT, 6], F32)    # bn_stats out
            me = singles.tile([GP, G, 2], F32)         # per-partition mean/var
            me2 = singles.tile([GP, G, 2], F32)        # (mean, E[x^2])
            t_sb = singles.tile([G, LT, LTS], BF16)    # t[g, l]

            gs_ps = gs_pool.tile([G, 2], F32)          # per-group mean, E[x^2]

            for lt in range(LT):
                sl = slice(lt * LTS, (lt + 1) * LTS)
                nc.sync.dma_start(out=x_sb[:, :, sl], in_=x_src[:, :, sl])
                # SWDGE cast copy: exact fp32 strip -> bf16 for the PE
                xb16 = xb16pool.tile([GP, G, LTS], BF16)
                nc.gpsimd.dma_start(out=xb16[:], in_=x_sb[:, :, sl])
                t_ps = t_pool.tile([G, LTS], F32)
                for g in range(G):
                    nc.vector.bn_stats(out=st6[:, g, lt, :], in_=x_sb[:, g, sl])
                    nc.tensor.matmul(
                        t_ps[:], lhsT=ugm_sb[:, :, g], rhs=xb16[:, g, :],
                        start=(g == 0), stop=(g == G - 1),
                    )
                nc.scalar.copy(t_sb[:, lt, :], t_ps[:])

            # per-partition -> per-group stats
            for g in range(G):
                nc.vector.bn_aggr(out=me[:, g, :], in_=st6[:, g, :, :])
                nc.vector.tensor_copy(me2[:, g, 0:1], me[:, g, 0:1])
                nc.vector.tensor_mul(me2[:, g, 1:2], me[:, g, 0:1], me[:, g, 0:1])
                nc.vector.tensor_add(me2[:, g, 1:2], me[:, g, 1:2], me2[:, g, 1:2])
                nc.tensor.matmul(gs_ps[:], lhsT=invm[:, :, g], rhs=me2[:, g, :],
                                 start=(g == 0), stop=(g == G - 1))

            # rsig_g = 1/sqrt(E[x^2] - mu^2 + eps)  on 8 partitions
            gs_sb = singles.tile([G, 2], F32)
            mu2 = singles.tile([G, 1], F32)
            varg = singles.tile([G, 1], F32)
            sd8 = singles.tile([G, 1], F32)
            rsig8 = singles.tile([G, 1], F32)
            rsig8b = singles.tile([G, 1], BF16)
            nc.scalar.copy(gs_sb[:], gs_ps[:])
            nc.vector.tensor_mul(mu2[:], gs_sb[:, 0:1], gs_sb[:, 0:1])
            nc.vector.tensor_sub(varg[:], gs_sb[:, 1:2], mu2[:])
            nc.scalar.activation(sd8[:], varg[:], mybir.ActivationFunctionType.Sqrt,
                                 bias=eps8[:], scale=1.0)
            nc.vector.reciprocal(rsig8[:], sd8[:])
            nc.scalar.copy(rsig8b[:], rsig8[:])

            # s[l] = sum_g rsig_g t[g,l]; e = exp(s/32); Z = sum e; attn = e/Z
            erow = singles.tile([1, L], F32)
            zrow = singles.tile([1, LT], F32)
            for lt in range(LT):
                s_ps = s_pool.tile([1, LTS], F32)
                nc.tensor.matmul(s_ps[:], lhsT=rsig8b[:], rhs=t_sb[:, lt, :],
                                 start=True, stop=True)
                nc.scalar.activation(erow[0:1, lt * LTS:(lt + 1) * LTS], s_ps[:],
                                     mybir.ActivationFunctionType.Exp,
                                     scale=SCALE, accum_out=zrow[0:1, lt:lt + 1])
            z1 = singles.tile([1, 1], F32)
            rz = singles.tile([1, 1], F32)
            attn = singles.tile([1, L], BF16)
            nc.vector.reduce_sum(out=z1[:], in_=zrow[:], axis=mybir.AxisListType.X)
            nc.vector.reciprocal(rz[:], z1[:])
            nc.vector.tensor_scalar_mul(attn[:], erow[:], rz[0:1, 0:1])

            # out = x + pv (x) attn + bp
            for g in range(G):
                for lt in range(LT):
                    sl = slice(lt * LTS, (lt + 1) * LTS)
                    o_ps = o_pool.tile([GP, LTS], F32)
                    nc.tensor.matmul(o_ps[:], lhsT=pv_sb[0:1, g, :], rhs=attn[0:1, sl],
                                     start=True, stop=True)
                    o_sb = obuf.tile([GP, LTS], F32)
                    nc.vector.scalar_tensor_tensor(
                        out=o_sb[:], in0=o_ps[:], scalar=bp_sb[:, g:g + 1],
                        in1=x_sb[:, g, sl],
                        op0=mybir.AluOpType.add, op1=mybir.AluOpType.add,
                    )
                    nc.sync.dma_start(out=out_dst[:, g, sl], in_=o_sb[:])

    nc.compile()
    return nc


def kernel(x, context, gn_gamma, gn_beta, ln_gamma, ln_beta,
           Wq, bq, Wkv, bkv, Wp, bp):
    x = np.asarray(x, np.float32)
    ctx = np.asarray(context, np.float64)
    # host-side context path (tiny): layernorm -> kv -> u = Wq^T k, pv = Wp v
    mu = ctx.mean(axis=1, keepdims=True)
    var = ctx.var(axis=1, keepdims=True)
    ctxn = (ctx - mu) / np.sqrt(var + EPS) * np.asarray(ln_gamma, np.float64) \
        + np.asarray(ln_beta, np.float64)
    kv = ctxn @ np.asarray(Wkv, np.float64).T + np.asarray(bkv, np.float64)
    k, v = kv[:, :C], kv[:, C:]
    u = k @ np.asarray(Wq, np.float64)               # u[b,c] = sum_o k[b,o] Wq[o,c]
    ug = u * np.asarray(gn_gamma, np.float64)[None, :]
    pv = v @ np.asarray(Wp, np.float64).T            # pv[b,o] = sum_c Wp[o,c] v[b,c]

    if "nc" not in _CACHE:
        _CACHE["nc"] = _build()
    nc = _CACHE["nc"]

    bf16 = ml_dtypes.bfloat16
    bpc = np.ascontiguousarray(
        np.asarray(bp, np.float32).reshape(G, GP).T)          # [p, g]
    in_maps = []
    for b in range(B):
        ugb = ug[b].astype(bf16).reshape(G, GP)                # [g, p]
        ugmask = np.zeros((GP, G, G), bf16)                    # [p, m, g]
        for g in range(G):
            ugmask[:, g, g] = ugb[g]
        in_maps.append({
            "x": np.ascontiguousarray(x[b]),
            "ugmask": ugmask,
            "pvr": np.ascontiguousarray(pv[b].astype(bf16)),
            "bpc": bpc,
        })
    global _LAST_IN_MAPS
    _LAST_IN_MAPS = in_maps
    res = run_bass_kernel_spmd(nc, in_maps, list(range(B)))
    return np.stack([res.results[b]["out"] for b in range(B)]).astype(np.float32)
